# revision 2
# baseline (speedup 1.0000x reference)
"""Trainium2 Bass kernel for nn_Attention (B,H,W,n,dim)=(2,64,64,8,512), 8 heads x 64.

Strategy (per core, 1024 pixels = 8192 rows of (pixel,token)):
  - row-tiles of 128 rows (16 pixels); 64 tiles per core.
  - x tile (bf16, host-cast) -> PE-transpose -> xT chunks [dim-part, rows].
  - QKV: q,k produced TRANSPOSED (stationary = w chunk, moving = xT) as 8
    feature-blocks [128=(2 heads x 64d), 128 rows]; v produced NATURAL
    (stationary = xT chunk, moving = w_v [128,512]).
  - scores: per (head h, 4-pixel group g): matmul(lhsT=qT[64d,32rows],
    rhs=kT[64d,32rows], tile_position=(64*(h%2), 32g)) -> psum [32,32]
    (cross-pixel garbage off the 8x8 diagonal blocks).
  - softmax on [128,256] tiles: TT-mult by host-built mask' (mask*SCALE at
    valid slots, 0 at garbage), exp (garbage -> 1), windowed reduce(8),
    reciprocal, TT-mult broadcast, TT-mult block-mask (zeros garbage, casts
    bf16), DVE 32x32 block transpose -> blockdiag attn^T.
  - AV: matmul(lhsT=attnT[32,32] blockdiag, rhs=v[32 rows, 64d of head],
    tile_position=(32g,32g)) -> av natural [128 rows, 512].
  - av -> PE-transpose -> avT; out-proj (stationary avT chunk, moving w_out
    chunk [128,512]) -> out [128 rows, 512] fp32 -> DMA out.
"""

import os
import sys

sys.path.insert(0, "/opt/trn_rl_repo")

import numpy as np
import ml_dtypes

import concourse.bass as bass
import concourse.bacc as bacc
import concourse.mybir as mybir
import concourse.tile as tile
from concourse.bass_utils import run_bass_kernel_spmd

HEADS = 8
DIM_HEAD = 64
SCALE = DIM_HEAD ** (-0.5)
B, H, W, NTOK, DIM = 2, 64, 64, 8, 512
INNER = HEADS * DIM_HEAD  # 512
N_CORES = 8
PIX_TOTAL = B * H * W          # 8192
PIX_CORE = PIX_TOTAL // N_CORES  # 1024
ROWS = PIX_CORE * NTOK         # 8192 rows per core
RT = 128                       # rows per tile (16 pixels)
NT = ROWS // RT                # 64 tiles
NT_BUILD = int(os.environ.get("KERNEL_NT", NT))  # reduced build for sim tests

BF16 = mybir.dt.bfloat16
F32 = mybir.dt.float32

_cache = {}


def build_nc(nt=NT_BUILD):
    nc = bacc.Bacc()
    x_d = nc.declare_dram_parameter("x", [ROWS, DIM], BF16, isOutput=False)
    wqkv_d = nc.declare_dram_parameter("wqkv", [DIM, 3 * INNER], BF16, isOutput=False)
    wout_d = nc.declare_dram_parameter("wout", [INNER, DIM], BF16, isOutput=False)
    maskp_d = nc.declare_dram_parameter("maskp", [ROWS, 128], BF16, isOutput=False)
    bmask_d = nc.declare_dram_parameter("bmask", [128, 128], BF16, isOutput=False)
    ident_d = nc.declare_dram_parameter("ident", [128, 128], BF16, isOutput=False)
    out_d = nc.declare_dram_parameter("out", [ROWS, DIM], F32, isOutput=True)

    MULT = mybir.AluOpType.mult
    ADD = mybir.AluOpType.add
    AXX = mybir.AxisListType.X

    with tile.TileContext(nc) as tc:
        with (
            tc.tile_pool(name="const", bufs=1) as constp,
            tc.tile_pool(name="xt", bufs=2) as xtp,
            tc.tile_pool(name="qkt", bufs=2) as qktp,
            tc.tile_pool(name="vsb", bufs=2) as vsbp,
            tc.tile_pool(name="sfx", bufs=2) as sfxp,
            tc.tile_pool(name="att", bufs=2) as attp,
            tc.tile_pool(name="avs", bufs=2) as avsp,
            tc.tile_pool(name="osb", bufs=8) as osbp,
            tc.tile_pool(name="ps_tp", bufs=1, space="PSUM") as ps_tp,
            tc.tile_pool(name="ps_qk", bufs=2, space="PSUM") as ps_qk,
            tc.tile_pool(name="ps_v", bufs=1, space="PSUM") as ps_v,
            tc.tile_pool(name="ps_sc", bufs=2, space="PSUM") as ps_sc,
            tc.tile_pool(name="ps_av", bufs=1, space="PSUM") as ps_av,
        ):
            # resident constants
            wq_sb = constp.tile([128, 4 * 1536], BF16, name="wq_sb")
            wo_sb = constp.tile([128, 4 * 512], BF16, name="wo_sb")
            ident = constp.tile([128, 128], BF16, name="ident_sb")
            bmask = constp.tile([128, 128], BF16, name="bmask_sb")
            for c in range(4):
                nc.sync.dma_start(out=wq_sb[:, c * 1536:(c + 1) * 1536],
                                  in_=wqkv_d[c * 128:(c + 1) * 128, :])
                nc.sync.dma_start(out=wo_sb[:, c * 512:(c + 1) * 512],
                                  in_=wout_d[c * 128:(c + 1) * 128, :])
            nc.sync.dma_start(out=ident[:], in_=ident_d[:])
            nc.sync.dma_start(out=bmask[:], in_=bmask_d[:])
            # pre-stage full x shard and mask' in SBUF (no per-tile load DMAs,
            # which sidesteps the 1-sync-wait limit on SWDGE DMA descriptors)
            xfull = constp.tile([128, NT * DIM], BF16, name="xfull")
            mfull = constp.tile([128, NT * 128], BF16, name="mfull")
            for tt in range(NT):
                nc.sync.dma_start(out=xfull[:, tt * DIM:(tt + 1) * DIM],
                                  in_=x_d[tt * RT:(tt + 1) * RT, :])
                nc.sync.dma_start(out=mfull[:, tt * 128:(tt + 1) * 128],
                                  in_=maskp_d[tt * RT:(tt + 1) * RT, :])

            for t in range(nt):
                r0 = t * RT
                xbf = xfull[:, t * DIM:(t + 1) * DIM]
                mkp = mfull[:, t * 128:(t + 1) * 128]

                # ---- transpose x -> xT (4 chunks of [128 dim, 128 rows])
                xT = xtp.tile([128, DIM], BF16, tag="xT")
                for c in range(4):
                    tp = ps_tp.tile([128, 128], BF16, tag="tp")
                    nc.tensor.transpose(tp[:], xbf[:, c * 128:(c + 1) * 128], ident[:])
                    if c % 2 == 0:
                        nc.scalar.copy(out=xT[:, c * 128:(c + 1) * 128], in_=tp[:])
                    else:
                        nc.vector.tensor_copy(out=xT[:, c * 128:(c + 1) * 128], in_=tp[:])

                # ---- q,k transposed GEMM: 8 feature blocks [128 feat, 128 rows]
                qkT = qktp.tile([128, 1024], BF16, tag="qkT_sb")
                for fb in range(8):
                    wcol = fb * 128 if fb < 4 else 512 + (fb - 4) * 128
                    qk_ps = ps_qk.tile([128, 128], F32, tag="qkfb")
                    for c in range(4):
                        nc.tensor.matmul(
                            qk_ps[:],
                            lhsT=wq_sb[:, c * 1536 + wcol: c * 1536 + wcol + 128],
                            rhs=xT[:, c * 128:(c + 1) * 128],
                            start=(c == 0), stop=(c == 3),
                        )
                    if fb % 2 == 0:
                        nc.scalar.copy(out=qkT[:, fb * 128:(fb + 1) * 128], in_=qk_ps[:])
                    else:
                        nc.vector.tensor_copy(out=qkT[:, fb * 128:(fb + 1) * 128], in_=qk_ps[:])
                # ---- v natural GEMM [128 rows, 512]
                v_ps = ps_v.tile([128, 512], F32, tag="v_ps")
                for c in range(4):
                    nc.tensor.matmul(
                        v_ps[:],
                        lhsT=xT[:, c * 128:(c + 1) * 128],
                        rhs=wq_sb[:, c * 1536 + 1024: c * 1536 + 1536],
                        start=(c == 0), stop=(c == 3),
                    )
                v_sb = vsbp.tile([128, 512], BF16, tag="v_sb")
                nc.scalar.copy(out=v_sb[:], in_=v_ps[:])

                # ---- scores per head [128,128] + mask-mult straight out of psum
                sm = sfxp.tile([128, 1024], BF16, tag="sm")
                for h in range(HEADS):
                    pb = 64 * (h % 2)
                    qof = (h // 2) * 128
                    kof = (4 + h // 2) * 128
                    sc_h = ps_sc.tile([128, 128], F32, tag="sch")
                    nc.tensor.matmul(
                        sc_h[:],
                        lhsT=qkT[pb:pb + 64, qof:qof + 128],
                        rhs=qkT[pb:pb + 64, kof:kof + 128],
                        start=True, stop=True,
                    )
                    nc.vector.tensor_tensor(out=sm[:, 128 * h:128 * h + 128],
                                            in0=sc_h[:], in1=mkp, op=MULT)
                ex = sfxp.tile([128, 1024], BF16, tag="ex")
                nc.scalar.activation(ex[:], sm[:], mybir.ActivationFunctionType.Exp)
                sums = sfxp.tile([128, 8], F32, tag="sums")
                nc.vector.tensor_reduce(
                    out=sums[:], in_=ex[:].rearrange("p (h c) -> p h c", h=8),
                    axis=AXX, op=ADD,
                )
                sumsc = sfxp.tile([128, 8], F32, tag="sumsc")
                nc.vector.tensor_scalar_add(sumsc[:], sums[:], -120.0)
                rec = sfxp.tile([128, 8], F32, tag="rec")
                nc.vector.reciprocal(rec[:], sumsc[:])
                attnb = attp.tile([128, 1024], BF16, tag="attnb")
                for h in range(HEADS):
                    hb = slice(128 * h, 128 * h + 128)
                    nc.vector.scalar_tensor_tensor(
                        out=attnb[:, hb], in0=ex[:, hb], scalar=rec[:, h:h + 1],
                        in1=bmask[:], op0=MULT, op1=MULT,
                    )
                attnT = attp.tile([128, 1024], BF16, tag="attnT")
                nc.vector.transpose(attnT[:], attnb[:])

                # ---- AV: per head full-array blockdiag attnT
                av_sb = avsp.tile([128, 512], BF16, tag="av_sb")
                for h in range(HEADS):
                    av_h = ps_av.tile([128, 64], F32, tag="avh")
                    nc.tensor.matmul(
                        av_h[:],
                        lhsT=attnT[:, 128 * h:128 * h + 128],
                        rhs=v_sb[:, 64 * h:64 * h + 64],
                        start=True, stop=True,
                    )
                    if h % 2 == 0:
                        nc.scalar.copy(out=av_sb[:, 64 * h:64 * h + 64], in_=av_h[:])
                    else:
                        nc.vector.tensor_copy(out=av_sb[:, 64 * h:64 * h + 64], in_=av_h[:])
                # ---- transpose av -> avT
                avT = avsp.tile([128, 512], BF16, tag="avT")
                for c in range(4):
                    tp2 = ps_tp.tile([128, 128], BF16, tag="tp")
                    nc.tensor.transpose(tp2[:], av_sb[:, c * 128:(c + 1) * 128], ident[:])
                    if c % 2 == 0:
                        nc.scalar.copy(out=avT[:, c * 128:(c + 1) * 128], in_=tp2[:])
                    else:
                        nc.vector.tensor_copy(out=avT[:, c * 128:(c + 1) * 128], in_=tp2[:])

                # ---- output projection [128 rows, 512]
                o_ps = ps_v.tile([128, 512], F32, tag="v_ps")
                for c in range(4):
                    nc.tensor.matmul(
                        o_ps[:],
                        lhsT=avT[:, c * 128:(c + 1) * 128],
                        rhs=wo_sb[:, c * 512:(c + 1) * 512],
                        start=(c == 0), stop=(c == 3),
                    )
                o_sb = osbp.tile([128, 512], F32, tag="o_sb")
                nc.scalar.copy(out=o_sb[:], in_=o_ps[:])
                nc.scalar.dma_start(out=out_d[r0:r0 + RT, :], in_=o_sb[:])
    return nc


def host_inputs(x, mask, w_qkv, w_out):
    """Build per-core input maps (host-side layout/dtype prep only)."""
    bf = ml_dtypes.bfloat16
    x_rows = np.ascontiguousarray(x.reshape(PIX_TOTAL * NTOK, DIM)).astype(bf)
    wq = np.ascontiguousarray(w_qkv).astype(bf)
    wo = np.ascontiguousarray(w_out).astype(bf)
    ident = np.eye(128, dtype=bf)

    # block indicator bmask[8*px + i, 8*px2 + j] = (px == px2), px over 16
    bm = np.zeros((16, 8, 16, 8), np.float32)
    for p in range(16):
        bm[p, :, p, :] = 1.0
    bmask = bm.reshape(128, 128).astype(bf)

    # mask' per row (px,i): cols (px'', j) = (px''==px%16) ? mask[px,i,j]*SCALE : 0
    m = mask.reshape(PIX_TOTAL, NTOK, NTOK).astype(np.float32)
    in_maps = []
    for cidx in range(N_CORES):
        mc = m[cidx * PIX_CORE:(cidx + 1) * PIX_CORE]  # [1024, 8, 8]
        mp = np.zeros((PIX_CORE, NTOK, 16, NTOK), np.float32)
        pl = np.arange(PIX_CORE) % 16
        mp[np.arange(PIX_CORE), :, pl, :] = mc * SCALE
        maskp = mp.reshape(ROWS, 128).astype(bf)
        in_maps.append({
            "x": x_rows[cidx * ROWS:(cidx + 1) * ROWS],
            "wqkv": wq,
            "wout": wo,
            "maskp": maskp,
            "bmask": bmask,
            "ident": ident,
        })
    return in_maps


def kernel(x, mask, w_qkv, w_out, b_out):
    if "nc" not in _cache:
        nc0 = build_nc(NT)
        nc0.finalize()
        _cache["nc"] = nc0
    nc = _cache["nc"]
    in_maps = host_inputs(x, mask, w_qkv, w_out)
    res = run_bass_kernel_spmd(nc, in_maps, list(range(N_CORES)))
    _cache["last_res"] = res
    outs = [np.asarray(r["out"], dtype=np.float32) for r in res.results]
    full = np.concatenate(outs, axis=0)  # [65536, 512]
    out = full.reshape(B, H, W, NTOK, DIM) + np.asarray(b_out, dtype=np.float32)
    return out.astype(np.float32)

